# revision 19
# baseline (speedup 1.0000x reference)
"""EdgeDegreeEmbedding Trainium2 kernel (8 NeuronCores, SPMD, no collectives).

Strategy: shard by TARGET NODE (625 nodes/core). Host sorts edges by target
node and packs each node's first 16 edges into a 16-row "half"; two halves
form a 32-partition-aligned slot, 8 halves form a 128-edge MLP tile with no
padding columns. A node's message sum is computed by PSUM-accumulated
matmuls whose stationary operand is a [32,*] slice of the MLP output m0 and
whose moving operand is a host-built block-diagonal wigner slice (envelope/
RESCALE pre-folded) - the edge->node scatter-add happens inside the PE.
Nodes with >16 edges spill into overflow halves added back on the host.
Each core only touches its private node range -> no allreduce.

v3: six-stage software pipeline (A | LN1+tr1 | D | LN2+tr2 | G+cast | R+out)
so every engine-queue instruction depends only on work from >=1 iteration
earlier. Layer-1 and the rotation run in fp8e4 DoubleRow mode (2 K-tiles per
matmul, 2x PE throughput) with host-side rescales (W1*16 cancels inside LN;
wigner*64 undone in the host unpack). The h transposes between layers use
the DMA XBAR instead of the PE. The two LayerNorm rsqrt Newton chains of an
iteration are batched into single [128,2] ops. x residual added on host.
"""

import numpy as np

import concourse.bass as bass
import concourse.mybir as mybir
from concourse import tile
from concourse.bass_utils import run_bass_kernel_spmd
from concourse.vector_clock import ScopedClock

# ---- problem constants (hardcoded; must match the reference) ----
SPHERE = 128
M0 = 7
LFULL = 49
CUTOFF = 12.0
RESCALE = 23.395238876342773
LN_EPS = 1e-5
N_NODES, N_EDGES, D_DIST = 5000, 50000, 512

N_CORES = 8
NODES_PER_CORE = N_NODES // N_CORES  # 625
SLOT_E = 32               # edge rows per slot (PE row band)
GSL = 3                   # node col-groups per slot
TILE_E = 128              # 4 slots * 32 edges per tile
SCOLS = GSL * LFULL       # 147: output cols per slot
HCOLS = 2 * SCOLS         # 294: output cols per 64-row half (2 slots)
WCOLS = M0 * HCOLS        # 2058: wigner section per tile row (2-pos)
XWF = 4 * 128 + WCOLS     # 512 + 2058 = 2570
OUTF = 4 * SCOLS          # 588 output cols per tile
RMAGIC = 0x5F3759DF
WSCALE = 64.0             # fp8 range shift for wigner; undone on host
W1SCALE = 16.0            # fp8 range shift for W1; cancels inside LN1

BF16 = mybir.dt.bfloat16
F32 = mybir.dt.float32
I32 = mybir.dt.int32
FP8 = mybir.dt.float8e4
NP_BF16 = mybir.dt.np(BF16)
NP_FP8 = mybir.dt.np(FP8)

_CACHE = {}
TRACE = False      # set True (e.g. from test.py) to profile the run
TRACE_KW = {}      # extra kwargs for run_bass_kernel_spmd when tracing
LAST = None        # BassKernelResults of the most recent run


class _ChunkedDrainTC(tile.TileContext):
    """Walrus here rejects >1 sync wait per instruction; spread every
    multi-wait instruction's extras over preceding same-engine nops, and do
    the same for the Tile exit-drain's global-clock waits."""

    def _lower_ordered_insts(self, ordered):
        for bb_name, insts in ordered.items():
            out = []
            for inst in insts:
                si = getattr(inst, "sync_info", None)
                waits = list(si.on_wait) if si is not None and si.on_wait else []
                if len(waits) > 1 and type(inst).__name__.startswith("Inst"):
                    for w in waits[:-1]:
                        out.append(mybir.InstNoOp(
                            name=self.nc.get_next_instruction_name(),
                            sync_info=mybir.SyncInfo(on_wait=[w], on_update=[]),
                            bass_nofuse=True,
                            engine=inst.engine,
                        ))
                    si.on_wait = waits[-1:]
                out.append(inst)
            ordered[bb_name] = out
        return super()._lower_ordered_insts(ordered)

    def _drain_and_barrier(self, tick_clock, wait_clock):
        nc = self.nc
        probe = nc.sync.nop()
        wait_clock.add_sem_waits(
            probe.ins, ScopedClock({None: tick_clock.global_clock})
        )
        si = probe.ins.sync_info
        waits = list(si.on_wait) if si and si.on_wait else []
        si.on_wait = waits[:1]
        for w in waits[1:]:
            n2 = nc.sync.nop()
            n2.ins.sync_info = mybir.SyncInfo(on_wait=[w], on_update=[])
        nc.sync.drain()
        nc.all_engine_barrier()
        popped = nc._tile_sem_poison_stack.pop()
        assert popped is self._sem_poison
        nc.clear_and_free_semaphores(list(self.sems.allocated().values()))
        nc.all_engine_barrier()


def _build_nc(T):
    """SPMD Bass program for T tiles (T_MAIN main + overflow). Six-stage
    pipeline; iteration i handles stage A for tile i, LN1 for i-1, D for
    i-2, LN2 for i-3, G+cast for i-4 and R+output for i-5."""
    nc = bass.Bass("TRN2", target_bir_lowering=False, num_devices=N_CORES)

    xw = nc.dram_tensor("xw", [T, 128, XWF], FP8, kind="ExternalInput")
    su = nc.dram_tensor("su", [T, 128, 128], BF16, kind="ExternalInput")
    w1 = nc.dram_tensor("w1", [128, 4 * 128], FP8, kind="ExternalInput")
    w2 = nc.dram_tensor("w2", [128, 128], BF16, kind="ExternalInput")
    w3 = nc.dram_tensor("w3", [128, M0 * SPHERE], BF16, kind="ExternalInput")
    ident = nc.dram_tensor("ident", [128, 128], BF16, kind="ExternalInput")

    outr = nc.dram_tensor("outr", [T, 128, OUTF], BF16, kind="ExternalOutput")

    DR = mybir.MatmulPerfMode.DoubleRow

    with _ChunkedDrainTC(nc) as tc:
        with (
            tc.tile_pool(name="const", bufs=1) as cpool,
            tc.tile_pool(name="xw", bufs=11) as xw_pool,
            tc.tile_pool(name="h", bufs=4) as h_pool,
            tc.tile_pool(name="ht", bufs=4) as ht_pool,
            tc.tile_pool(name="m0", bufs=3) as m0_pool,
            tc.tile_pool(name="outt", bufs=4) as out_pool,
            tc.tile_pool(name="stat", bufs=4) as stat_pool,
            tc.tile_pool(name="ps1", bufs=2, space="PSUM") as ps1_pool,
            tc.tile_pool(name="ps2", bufs=2, space="PSUM") as ps2_pool,
            tc.tile_pool(name="pst", bufs=1, space="PSUM") as pst_pool,
            tc.tile_pool(name="m0ps", bufs=1, space="PSUM") as m0ps_pool,
            tc.tile_pool(name="psr", bufs=2, space="PSUM") as psr_pool,
        ):
            w1_sb = cpool.tile([128, 4 * 128], FP8)
            nc.sync.dma_start(w1_sb[:], w1[:])
            w2_sb = cpool.tile([128, 128], BF16)
            nc.sync.dma_start(w2_sb[:], w2[:])
            w3_sb = cpool.tile([128, M0 * SPHERE], BF16)
            nc.sync.dma_start(w3_sb[:], w3[:])
            id_sb = cpool.tile([128, 128], BF16)
            nc.sync.dma_start(id_sb[:], ident[:])
            epsT = cpool.tile([128, 2], F32)
            nc.vector.memset(epsT[:], LN_EPS)
            c15T = cpool.tile([128, 2], F32)
            nc.vector.memset(c15T[:], 1.5)
            nhT = cpool.tile([128, 2], F32)
            nc.vector.memset(nhT[:], -0.5)
            zero2 = cpool.tile([128, 2], F32)
            nc.vector.memset(zero2[:], 0.0)

            live = {}

            def stats(ps, mv4, lo):
                """bn stats of one [128,128] psum into mv4[:, lo:lo+2]."""
                st = stat_pool.tile([128, 6], F32, tag=f"bn{lo}")
                nc.vector.bn_stats(st[:], ps)
                nc.vector.bn_aggr(mv4[:, lo:lo + 2], st[:])

            def rot_half(t, h):
                """One 64-row half (2 slots) of the rotation for tile t:
                3 fp8 DoubleRow matmuls (m-pairs) + 1 plain fp8 matmul (m=6),
                K=64 each, accumulated into a [128,294] psum whose columns
                are the two slots' group-diagonal blocks (host-built), then
                one cast-copy to out_sb (scalar for h=0, vector for h=1)."""
                st_ = live[t]
                xw_t, m0_sb, out_sb = st_["xw"], st_["m0"], st_["out"]
                hb = 64 * h
                rot = psr_pool.tile([128, HCOLS], F32, tag="rot")
                for j in range(3):
                    lhs = m0_sb[hb:hb + 64, j * 256:(j + 1) * 256].rearrange(
                        "p (j c) -> p j c", j=2)
                    rhs = xw_t[hb:hb + 64,
                               512 + 2 * j * HCOLS:
                               512 + (2 * j + 2) * HCOLS].rearrange(
                        "p (j f) -> p j f", j=2)
                    nc.tensor.matmul(rot[:], lhs, rhs,
                                     start=(j == 0), stop=False,
                                     perf_mode=DR, tile_position=(hb, 0))
                nc.tensor.matmul(
                    rot[:],
                    m0_sb[hb:hb + 64, 6 * 128:7 * 128],
                    xw_t[hb:hb + 64, 512 + 6 * HCOLS:512 + 7 * HCOLS],
                    start=False, stop=True, tile_position=(hb, 0),
                )
                dst = out_sb[:, h * HCOLS:(h + 1) * HCOLS]
                if h == 0:
                    nc.scalar.activation(dst, rot[:],
                                         mybir.ActivationFunctionType.Copy)
                else:
                    nc.vector.tensor_copy(dst, rot[:])

            for i in range(T + 10):
                ta, t1_, tb, t2_, td, tg, te, tf = (
                    i, i - 1, i - 3, i - 4, i - 6, i - 7, i - 8, i - 9)

                # ---- xw prefetch (sync queue), 2 iterations ahead ----
                pf = [0, 1, 2] if i == 0 else [i + 2]
                for tp in pf:
                    if tp < T:
                        xw_t = xw_pool.tile([128, XWF], FP8, name="xw_t",
                                            tag="xw_t")
                        nc.sync.dma_start(xw_t[:], xw[tp])
                        su_t = xw_pool.tile([128, 128], BF16, name="su_t",
                                            tag="su_t")
                        nc.sync.dma_start(su_t[:], su[tp])
                        live[tp] = {"xw": xw_t, "su": su_t}

                # ---- stage A(ta): layer-1, 3 fp8 DoubleRow matmuls ----
                if ta < T:
                    st_ = live[ta]
                    ps1 = ps1_pool.tile([128, 128], F32, tag="ps1")
                    for j in range(2):
                        lhs = st_["xw"][:, j * 256:(j + 1) * 256].rearrange(
                            "p (j e) -> p j e", j=2)
                        rhs = w1_sb[:, j * 256:(j + 1) * 256].rearrange(
                            "p (j c) -> p j c", j=2)
                        nc.tensor.matmul(ps1[:], lhs, rhs,
                                         start=(j == 0), stop=(j == 1),
                                         perf_mode=DR)
                    nc.vector.tensor_add(ps1[:], ps1[:], st_["su"][:])
                    st_["ps1"] = ps1

                # ---- LN stats: LN1(t1_) + LN2(t2_) ----
                mv4 = stat_pool.tile([128, 4], F32, tag="mv4")
                if 0 <= t1_ < T:
                    stats(live[t1_]["ps1"][:], mv4, 0)

                # tr1 for tile i-2: silu1(i-2) ran last iteration
                if 0 <= i - 2 < T:
                    st_ = live[i - 2]
                    pst1 = pst_pool.tile([128, 128], BF16, tag="pst")
                    nc.tensor.transpose(pst1[:], st_["h1"][:], id_sb[:])
                    h1t = ht_pool.tile([128, 128], BF16, tag="h1t")
                    nc.vector.tensor_copy(h1t[:], pst1[:])
                    st_["h1t"] = h1t

                if 0 <= te < T:
                    rot_half(te, 0)

                # ---- stage D(tb): layer-2 matmul (bf16) ----
                if 0 <= tb < T:
                    st_ = live[tb]
                    ps2 = ps2_pool.tile([128, 128], F32, tag="ps2")
                    nc.tensor.matmul(ps2[:], st_["h1t"][:], w2_sb[:],
                                     start=True, stop=True)
                    st_["ps2"] = ps2

                if 0 <= t2_ < T:
                    stats(live[t2_]["ps2"][:], mv4, 2)

                # ---- batched rsqrt Newton chain on [128,2] columns ----
                any_ln = (0 <= t1_ < T) or (0 <= t2_ < T)
                if any_ln:
                    ve = stat_pool.tile([128, 2], F32, tag="ve")
                    nc.gpsimd.tensor_add(ve[:], mv4[:, 1:4:2], epsT[:])
                    yi = stat_pool.tile([128, 2], I32, tag="yi")
                    yf = yi[:].bitcast(F32)
                    nc.vector.tensor_scalar(yi[:], ve[:].bitcast(I32), 1,
                                            None,
                                            mybir.AluOpType.arith_shift_right)
                    nc.vector.tensor_scalar(yi[:], yi[:], -1, RMAGIC,
                                            mybir.AluOpType.mult,
                                            mybir.AluOpType.add)
                    t1t = stat_pool.tile([128, 2], F32, tag="t1t")
                    nc.gpsimd.tensor_mul(t1t[:], yf, yf)
                    nc.gpsimd.tensor_mul(t1t[:], t1t[:], ve[:])
                    nc.gpsimd.tensor_mul(t1t[:], t1t[:], nhT[:])
                    nc.gpsimd.tensor_add(t1t[:], t1t[:], c15T[:])
                    nc.gpsimd.tensor_mul(yf, yf, t1t[:])
                    nm = stat_pool.tile([128, 2], F32, tag="nm")
                    nc.gpsimd.tensor_mul(nm[:], mv4[:, 0:3:2], yf)
                    nc.gpsimd.tensor_sub(nm[:], zero2[:], nm[:])

                # ---- silu for LN1(t1_) ----
                if 0 <= t1_ < T:
                    st_ = live[t1_]
                    h1 = h_pool.tile([128, 128], BF16, tag="h1")
                    nc.scalar.activation(h1[:], st_["ps1"][:],
                                         mybir.ActivationFunctionType.Silu,
                                         bias=nm[:, 0:1], scale=yf[:, 0:1])
                    st_["h1"] = h1

                # ---- stage Ga(td): layer-3 first half + fp8 cast ----
                if 0 <= td < T:
                    st_ = live[td]
                    m0a = m0ps_pool.tile([128, 448], F32, tag="m0ps",
                                         name="m0a")
                    nc.tensor.matmul(m0a[:], st_["h2t"][:], w3_sb[:, 0:448],
                                     start=True, stop=True)
                    m0_sb = m0_pool.tile([128, M0 * SPHERE], FP8)
                    nc.scalar.activation(m0_sb[:, 0:448], m0a[:],
                                         mybir.ActivationFunctionType.Copy)
                    st_["m0"] = m0_sb
                    st_["out"] = out_pool.tile([128, OUTF], BF16,
                                               name="out_sb", tag="out_sb")

                # ---- stage Gb(tg): layer-3 second half + fp8 cast ----
                if 0 <= tg < T:
                    st_ = live[tg]
                    m0b = m0ps_pool.tile([128, 448], F32, tag="m0ps",
                                         name="m0b")
                    nc.tensor.matmul(m0b[:], st_["h2t"][:], w3_sb[:, 448:896],
                                     start=True, stop=True)
                    nc.vector.tensor_copy(st_["m0"][:, 448:896], m0b[:])

                # ---- silu for LN2(t2_) ----
                if 0 <= t2_ < T:
                    st_ = live[t2_]
                    h2 = h_pool.tile([128, 128], BF16, tag="h2")
                    nc.scalar.activation(h2[:], st_["ps2"][:],
                                         mybir.ActivationFunctionType.Silu,
                                         bias=nm[:, 1:2], scale=yf[:, 1:2])
                    st_["h2"] = h2

                # tr2 for tile i-5: silu2(i-5) ran last iteration
                if 0 <= i - 5 < T:
                    st_ = live[i - 5]
                    pst2 = pst_pool.tile([128, 128], BF16, tag="pst")
                    nc.tensor.transpose(pst2[:], st_["h2"][:], id_sb[:])
                    h2t = ht_pool.tile([128, 128], BF16, tag="h2t")
                    nc.vector.tensor_copy(h2t[:], pst2[:])
                    st_["h2t"] = h2t

                if 0 <= te < T:
                    rot_half(te, 1)

                # ---- output DMA for tile tf: copies ran last iteration ----
                if 0 <= tf < T:
                    nc.gpsimd.dma_start(outr[tf], live[tf]["out"][:])
                    del live[tf]

    return nc


def _envelope(d):
    e = 1.0 + (-21.0) * d ** 5 + 35.0 * d ** 6 + (-15.0) * d ** 7
    return np.where(d < 1.0, e, 0.0)


def kernel(**inputs):
    x = np.asarray(inputs["x"], np.float32)
    dist_emb = np.asarray(inputs["edge_distance_embedding"], np.float32)
    src_emb = np.asarray(inputs["source_atom_embedding"], np.float32)
    tgt_emb = np.asarray(inputs["target_atom_embedding"], np.float32)
    edge_distance = np.asarray(inputs["edge_distance"], np.float64)
    edge_index = np.asarray(inputs["edge_index"]).astype(np.int64)
    wigner = np.asarray(inputs["wigner_and_M_mapping_inv"], np.float32)
    W1 = np.asarray(inputs["W1"], np.float32)
    W2 = np.asarray(inputs["W2"], np.float32)
    W3 = np.asarray(inputs["W3"], np.float32)
    # biases/gains are zeros/ones by construction; folded out of the kernel
    for nm, triv in (("b1", 0), ("bt1", 0), ("b2", 0), ("bt2", 0), ("b3", 0),
                     ("g1", 1), ("g2", 1)):
        v = np.asarray(inputs[nm])
        assert np.all(v == triv), f"{nm} not trivial; unsupported fast path"

    srcs, tgts = edge_index[0], edge_index[1]
    scale = (_envelope(edge_distance / CUTOFF) / RESCALE * WSCALE).astype(
        np.float32)

    order = np.argsort(tgts, kind="stable")
    tsorted = tgts[order]
    starts = np.searchsorted(tsorted, np.arange(N_NODES + 1))

    # ---- pack nodes into 32-edge slots, <=3 nodes (col-groups) each ----
    core_slots = []
    max_T = 0
    for c in range(N_CORES):
        base = c * NODES_PER_CORE
        pieces = []  # (local node, start offset into its edge list, count)
        for nl in range(NODES_PER_CORE):
            d = int(starts[base + nl + 1] - starts[base + nl])
            off = 0
            while d > SLOT_E:
                pieces.append((nl, off, SLOT_E))
                off += SLOT_E
                d -= SLOT_E
            pieces.append((nl, off, d))
        pieces.sort(key=lambda p: -p[2])
        slots = []  # [rem, [(nl, off, cnt), ...]]
        for nl, off, d in pieces:
            best, bestrem = -1, SLOT_E + 1
            for i, (rem, members) in enumerate(slots):
                if len(members) < GSL and d <= rem < bestrem:
                    best, bestrem = i, rem
            if best >= 0:
                slots[best][0] -= d
                slots[best][1].append((nl, off, d))
            else:
                slots.append([SLOT_E - d, [(nl, off, d)]])
        core_slots.append(slots)
        max_T = max(max_T, -(-len(slots) // 4))

    T = max_T
    n_slots = 4 * T
    E_pad = n_slots * SLOT_E

    if T not in _CACHE:
        _CACHE[T] = _build_nc(T)
    nc = _CACHE[T]

    # ---- shared weight tensors ----
    w1_in = np.ascontiguousarray(
        (W1[:D_DIST] * W1SCALE).reshape(4, 128, 128).transpose(1, 0, 2)
        .reshape(128, 4 * 128)
    ).astype(NP_FP8)
    su_proj = (src_emb @ (W1[D_DIST:D_DIST + 128] * W1SCALE))
    tu_proj = (tgt_emb @ (W1[D_DIST + 128:] * W1SCALE))
    w2_in = W2.astype(NP_BF16)
    w3_in = W3.astype(NP_BF16)
    ident = np.eye(128, dtype=np.float32).astype(NP_BF16)

    in_maps = []
    unpack_maps = []
    for c in range(N_CORES):
        base = c * NODES_PER_CORE
        slots = core_slots[c]

        eorder = np.full(E_pad, -1, np.int64)
        grp = np.zeros(E_pad, np.int64)
        ent_t, ent_s, ent_g, ent_n = [], [], [], []
        for si, (_, members) in enumerate(slots):
            r = si * SLOT_E
            for g, (nl, off, cnt) in enumerate(members):
                s0 = starts[base + nl] + off
                eorder[r:r + cnt] = order[s0:s0 + cnt]
                grp[r:r + cnt] = g
                ent_t.append(si // 4)
                ent_s.append(si % 4)
                ent_g.append(g)
                ent_n.append(nl)
                r += cnt
        valid = eorder >= 0
        idx = eorder[valid]

        # xe gather (dist-embedding part) -> [T, 128p, 4k*128e]
        xe = np.zeros((E_pad, D_DIST), np.float32)
        xe[valid] = dist_emb[idx]
        xeT = xe.reshape(T, TILE_E, 4, 128).transpose(0, 3, 2, 1)
        # per-edge precomputed src/tgt projection, [T, 128e, 128c]
        sue = np.zeros((E_pad, 128), np.float32)
        sue[valid] = su_proj[srcs[idx]] + tu_proj[tgts[idx]]
        su_in = np.ascontiguousarray(sue.reshape(T, TILE_E, 128)).astype(
            NP_BF16)

        # group-diagonal wigner: row r of slot s, col-group g:
        # xw[t, 32s+r, 768 + m*147 + g*49 + f] = wig[e,f,m]*scale
        wrows = np.zeros((E_pad, M0, LFULL), np.float32)
        wrows[valid] = (
            wigner[idx, :, :M0] * scale[idx][:, None, None]
        ).transpose(0, 2, 1)
        spos = (np.arange(E_pad) // SLOT_E) % 2
        wsec = np.zeros((E_pad, M0, 2, GSL, LFULL), np.float32)
        wsec[np.arange(E_pad), :, spos, grp, :] = wrows
        wsec = wsec.reshape(T, 128, WCOLS)

        xw_in = np.ascontiguousarray(np.concatenate(
            (xeT.reshape(T, 128, 512), wsec), axis=2,
        )).astype(NP_FP8)

        in_maps.append({
            "xw": xw_in, "su": su_in,
            "w1": w1_in, "w2": w2_in, "w3": w3_in, "ident": ident,
        })
        unpack_maps.append((np.asarray(ent_t), np.asarray(ent_s),
                            np.asarray(ent_g), np.asarray(ent_n)))

    global LAST
    res = run_bass_kernel_spmd(
        nc, in_maps, core_ids=list(range(N_CORES)), trace=TRACE, **TRACE_KW
    )
    LAST = res

    out = np.empty((N_NODES, LFULL, SPHERE), np.float32)
    inv_ws = np.float32(1.0 / WSCALE)
    for c in range(N_CORES):
        r = res.results[c]
        # [T, 128c, 4s, 3g, 49f] -> [T, 4, 3, 49, 128]
        o = (np.asarray(r["outr"], np.float32) * inv_ws).reshape(
            T, 128, 4, GSL, LFULL).transpose(0, 2, 3, 4, 1)
        tt, ss, gg, nn = unpack_maps[c]
        acc = np.zeros((NODES_PER_CORE, LFULL, SPHERE), np.float32)
        np.add.at(acc, nn, o[tt, ss, gg])
        out[c * NODES_PER_CORE:(c + 1) * NODES_PER_CORE] = (
            x[c * NODES_PER_CORE:(c + 1) * NODES_PER_CORE] + acc
        )
    return out


# revision 20
# speedup vs baseline: 1.0145x; 1.0145x over previous
"""EdgeDegreeEmbedding Trainium2 kernel (8 NeuronCores, SPMD, no collectives).

Strategy: shard by TARGET NODE (625 nodes/core). Host sorts edges by target
node and packs each node's first 16 edges into a 16-row "half"; two halves
form a 32-partition-aligned slot, 8 halves form a 128-edge MLP tile with no
padding columns. A node's message sum is computed by PSUM-accumulated
matmuls whose stationary operand is a [32,*] slice of the MLP output m0 and
whose moving operand is a host-built block-diagonal wigner slice (envelope/
RESCALE pre-folded) - the edge->node scatter-add happens inside the PE.
Nodes with >16 edges spill into overflow halves added back on the host.
Each core only touches its private node range -> no allreduce.

v3: six-stage software pipeline (A | LN1+tr1 | D | LN2+tr2 | G+cast | R+out)
so every engine-queue instruction depends only on work from >=1 iteration
earlier. Layer-1 and the rotation run in fp8e4 DoubleRow mode (2 K-tiles per
matmul, 2x PE throughput) with host-side rescales (W1*16 cancels inside LN;
wigner*64 undone in the host unpack). The h transposes between layers use
the DMA XBAR instead of the PE. The two LayerNorm rsqrt Newton chains of an
iteration are batched into single [128,2] ops. x residual added on host.
"""

import numpy as np

import concourse.bass as bass
import concourse.mybir as mybir
from concourse import tile
from concourse.bass_utils import run_bass_kernel_spmd
from concourse.vector_clock import ScopedClock

# ---- problem constants (hardcoded; must match the reference) ----
SPHERE = 128
M0 = 7
LFULL = 49
CUTOFF = 12.0
RESCALE = 23.395238876342773
LN_EPS = 1e-5
N_NODES, N_EDGES, D_DIST = 5000, 50000, 512

N_CORES = 8
NODES_PER_CORE = N_NODES // N_CORES  # 625
SLOT_E = 32               # edge rows per slot (PE row band)
GSL = 3                   # node col-groups per slot
TILE_E = 128              # 4 slots * 32 edges per tile
SCOLS = GSL * LFULL       # 147: output cols per slot
HCOLS = 2 * SCOLS         # 294: output cols per 64-row half (2 slots)
WCOLS = M0 * HCOLS        # 2058: wigner section per tile row (2-pos)
XWF = 4 * 128 + WCOLS     # 512 + 2058 = 2570
OUTF = 4 * SCOLS          # 588 output cols per tile
RMAGIC = 0x5F3759DF
WSCALE = 64.0             # fp8 range shift for wigner; undone on host
W1SCALE = 16.0            # fp8 range shift for W1; cancels inside LN1

BF16 = mybir.dt.bfloat16
F32 = mybir.dt.float32
I32 = mybir.dt.int32
FP8 = mybir.dt.float8e4
NP_BF16 = mybir.dt.np(BF16)
NP_FP8 = mybir.dt.np(FP8)

_CACHE = {}
TRACE = False      # set True (e.g. from test.py) to profile the run
TRACE_KW = {}      # extra kwargs for run_bass_kernel_spmd when tracing
LAST = None        # BassKernelResults of the most recent run


class _ChunkedDrainTC(tile.TileContext):
    """Walrus here rejects >1 sync wait per instruction; spread every
    multi-wait instruction's extras over preceding same-engine nops, and do
    the same for the Tile exit-drain's global-clock waits."""

    def _lower_ordered_insts(self, ordered):
        for bb_name, insts in ordered.items():
            out = []
            for inst in insts:
                si = getattr(inst, "sync_info", None)
                waits = list(si.on_wait) if si is not None and si.on_wait else []
                if len(waits) > 1 and type(inst).__name__.startswith("Inst"):
                    for w in waits[:-1]:
                        out.append(mybir.InstNoOp(
                            name=self.nc.get_next_instruction_name(),
                            sync_info=mybir.SyncInfo(on_wait=[w], on_update=[]),
                            bass_nofuse=True,
                            engine=inst.engine,
                        ))
                    si.on_wait = waits[-1:]
                out.append(inst)
            ordered[bb_name] = out
        return super()._lower_ordered_insts(ordered)

    def _drain_and_barrier(self, tick_clock, wait_clock):
        nc = self.nc
        probe = nc.sync.nop()
        wait_clock.add_sem_waits(
            probe.ins, ScopedClock({None: tick_clock.global_clock})
        )
        si = probe.ins.sync_info
        waits = list(si.on_wait) if si and si.on_wait else []
        si.on_wait = waits[:1]
        for w in waits[1:]:
            n2 = nc.sync.nop()
            n2.ins.sync_info = mybir.SyncInfo(on_wait=[w], on_update=[])
        nc.sync.drain()
        nc.all_engine_barrier()
        popped = nc._tile_sem_poison_stack.pop()
        assert popped is self._sem_poison
        nc.clear_and_free_semaphores(list(self.sems.allocated().values()))
        nc.all_engine_barrier()


def _build_nc(T):
    """SPMD Bass program for T tiles (T_MAIN main + overflow). Six-stage
    pipeline; iteration i handles stage A for tile i, LN1 for i-1, D for
    i-2, LN2 for i-3, G+cast for i-4 and R+output for i-5."""
    nc = bass.Bass("TRN2", target_bir_lowering=False, num_devices=N_CORES)

    xw = nc.dram_tensor("xw", [T, 128, XWF], FP8, kind="ExternalInput")
    su = nc.dram_tensor("su", [T, 128, 128], BF16, kind="ExternalInput")
    w1 = nc.dram_tensor("w1", [128, 4 * 128], FP8, kind="ExternalInput")
    w2 = nc.dram_tensor("w2", [128, 128], BF16, kind="ExternalInput")
    w3 = nc.dram_tensor("w3", [128, M0 * SPHERE], BF16, kind="ExternalInput")
    ident = nc.dram_tensor("ident", [128, 128], BF16, kind="ExternalInput")

    outr = nc.dram_tensor("outr", [T, 128, OUTF], BF16, kind="ExternalOutput")

    DR = mybir.MatmulPerfMode.DoubleRow

    with _ChunkedDrainTC(nc) as tc:
        with (
            tc.tile_pool(name="const", bufs=1) as cpool,
            tc.tile_pool(name="xw", bufs=11) as xw_pool,
            tc.tile_pool(name="h", bufs=4) as h_pool,
            tc.tile_pool(name="ht", bufs=4) as ht_pool,
            tc.tile_pool(name="m0", bufs=3) as m0_pool,
            tc.tile_pool(name="outt", bufs=4) as out_pool,
            tc.tile_pool(name="stat", bufs=4) as stat_pool,
            tc.tile_pool(name="ps1", bufs=2, space="PSUM") as ps1_pool,
            tc.tile_pool(name="ps2", bufs=2, space="PSUM") as ps2_pool,
            tc.tile_pool(name="pst", bufs=1, space="PSUM") as pst_pool,
            tc.tile_pool(name="m0ps", bufs=1, space="PSUM") as m0ps_pool,
            tc.tile_pool(name="psr", bufs=1, space="PSUM") as psr_pool,
        ):
            w1_sb = cpool.tile([128, 4 * 128], FP8)
            nc.sync.dma_start(w1_sb[:], w1[:])
            w2_sb = cpool.tile([128, 128], BF16)
            nc.sync.dma_start(w2_sb[:], w2[:])
            w3_sb = cpool.tile([128, M0 * SPHERE], BF16)
            nc.sync.dma_start(w3_sb[:], w3[:])
            id_sb = cpool.tile([128, 128], BF16)
            nc.sync.dma_start(id_sb[:], ident[:])
            epsT = cpool.tile([128, 2], F32)
            nc.vector.memset(epsT[:], LN_EPS)
            c15T = cpool.tile([128, 2], F32)
            nc.vector.memset(c15T[:], 1.5)
            nhT = cpool.tile([128, 2], F32)
            nc.vector.memset(nhT[:], -0.5)
            zero2 = cpool.tile([128, 2], F32)
            nc.vector.memset(zero2[:], 0.0)

            live = {}

            def stats(ps, mv4, lo):
                """bn stats of one [128,128] psum into mv4[:, lo:lo+2]."""
                st = stat_pool.tile([128, 6], F32, tag=f"bn{lo}")
                nc.vector.bn_stats(st[:], ps)
                nc.vector.bn_aggr(mv4[:, lo:lo + 2], st[:])

            def rot_half(t, h):
                """One 64-row half (2 slots) of the rotation for tile t:
                3 fp8 DoubleRow matmuls (m-pairs) + 1 plain fp8 matmul (m=6),
                K=64 each, accumulated into a [128,294] psum whose columns
                are the two slots' group-diagonal blocks (host-built), then
                one cast-copy to out_sb (scalar for h=0, vector for h=1)."""
                st_ = live[t]
                xw_t, m0_sb, out_sb = st_["xw"], st_["m0"], st_["out"]
                hb = 64 * h
                rot = psr_pool.tile([128, HCOLS], F32, tag="rot")
                for j in range(3):
                    lhs = m0_sb[hb:hb + 64, j * 256:(j + 1) * 256].rearrange(
                        "p (j c) -> p j c", j=2)
                    rhs = xw_t[hb:hb + 64,
                               512 + 2 * j * HCOLS:
                               512 + (2 * j + 2) * HCOLS].rearrange(
                        "p (j f) -> p j f", j=2)
                    nc.tensor.matmul(rot[:], lhs, rhs,
                                     start=(j == 0), stop=False,
                                     perf_mode=DR, tile_position=(hb, 0))
                nc.tensor.matmul(
                    rot[:],
                    m0_sb[hb:hb + 64, 6 * 128:7 * 128],
                    xw_t[hb:hb + 64, 512 + 6 * HCOLS:512 + 7 * HCOLS],
                    start=False, stop=True, tile_position=(hb, 0),
                )
                dst = out_sb[:, h * HCOLS:(h + 1) * HCOLS]
                if h == 0:
                    nc.scalar.activation(dst, rot[:],
                                         mybir.ActivationFunctionType.Copy)
                else:
                    nc.vector.tensor_copy(dst, rot[:])

            for i in range(T + 9):
                ta, t1_, tb, t2_, td, te, tf = (
                    i, i - 1, i - 3, i - 4, i - 6, i - 7, i - 8)

                # ---- xw prefetch (sync queue), 2 iterations ahead ----
                pf = [0, 1, 2] if i == 0 else [i + 2]
                for tp in pf:
                    if tp < T:
                        xw_t = xw_pool.tile([128, XWF], FP8, name="xw_t",
                                            tag="xw_t")
                        nc.sync.dma_start(xw_t[:], xw[tp])
                        su_t = xw_pool.tile([128, 128], BF16, name="su_t",
                                            tag="su_t")
                        nc.sync.dma_start(su_t[:], su[tp])
                        live[tp] = {"xw": xw_t, "su": su_t}

                # ---- stage A(ta): layer-1, 3 fp8 DoubleRow matmuls ----
                if ta < T:
                    st_ = live[ta]
                    ps1 = ps1_pool.tile([128, 128], F32, tag="ps1")
                    for j in range(2):
                        lhs = st_["xw"][:, j * 256:(j + 1) * 256].rearrange(
                            "p (j e) -> p j e", j=2)
                        rhs = w1_sb[:, j * 256:(j + 1) * 256].rearrange(
                            "p (j c) -> p j c", j=2)
                        nc.tensor.matmul(ps1[:], lhs, rhs,
                                         start=(j == 0), stop=(j == 1),
                                         perf_mode=DR)
                    nc.vector.tensor_add(ps1[:], ps1[:], st_["su"][:])
                    st_["ps1"] = ps1

                # ---- LN stats: LN1(t1_) + LN2(t2_) ----
                mv4 = stat_pool.tile([128, 4], F32, tag="mv4")
                if 0 <= t1_ < T:
                    stats(live[t1_]["ps1"][:], mv4, 0)

                # tr1 for tile i-2: silu1(i-2) ran last iteration
                if 0 <= i - 2 < T:
                    st_ = live[i - 2]
                    pst1 = pst_pool.tile([128, 128], BF16, tag="pst")
                    nc.tensor.transpose(pst1[:], st_["h1"][:], id_sb[:])
                    h1t = ht_pool.tile([128, 128], BF16, tag="h1t")
                    nc.vector.tensor_copy(h1t[:], pst1[:])
                    st_["h1t"] = h1t

                if 0 <= te < T:
                    rot_half(te, 0)

                # ---- stage D(tb): layer-2 matmul (bf16) ----
                if 0 <= tb < T:
                    st_ = live[tb]
                    ps2 = ps2_pool.tile([128, 128], F32, tag="ps2")
                    nc.tensor.matmul(ps2[:], st_["h1t"][:], w2_sb[:],
                                     start=True, stop=True)
                    st_["ps2"] = ps2

                if 0 <= t2_ < T:
                    stats(live[t2_]["ps2"][:], mv4, 2)

                # ---- batched rsqrt Newton chain on [128,2] columns ----
                any_ln = (0 <= t1_ < T) or (0 <= t2_ < T)
                if any_ln:
                    ve = stat_pool.tile([128, 2], F32, tag="ve")
                    nc.gpsimd.tensor_add(ve[:], mv4[:, 1:4:2], epsT[:])
                    yi = stat_pool.tile([128, 2], I32, tag="yi")
                    yf = yi[:].bitcast(F32)
                    nc.vector.tensor_scalar(yi[:], ve[:].bitcast(I32), 1,
                                            None,
                                            mybir.AluOpType.arith_shift_right)
                    nc.vector.tensor_scalar(yi[:], yi[:], -1, RMAGIC,
                                            mybir.AluOpType.mult,
                                            mybir.AluOpType.add)
                    t1t = stat_pool.tile([128, 2], F32, tag="t1t")
                    nc.gpsimd.tensor_mul(t1t[:], yf, yf)
                    nc.gpsimd.tensor_mul(t1t[:], t1t[:], ve[:])
                    nc.gpsimd.tensor_mul(t1t[:], t1t[:], nhT[:])
                    nc.gpsimd.tensor_add(t1t[:], t1t[:], c15T[:])
                    nc.gpsimd.tensor_mul(yf, yf, t1t[:])
                    nm = stat_pool.tile([128, 2], F32, tag="nm")
                    nc.gpsimd.tensor_mul(nm[:], mv4[:, 0:3:2], yf)
                    nc.gpsimd.tensor_sub(nm[:], zero2[:], nm[:])

                # ---- silu for LN1(t1_) ----
                if 0 <= t1_ < T:
                    st_ = live[t1_]
                    h1 = h_pool.tile([128, 128], BF16, tag="h1")
                    nc.scalar.activation(h1[:], st_["ps1"][:],
                                         mybir.ActivationFunctionType.Silu,
                                         bias=nm[:, 0:1], scale=yf[:, 0:1])
                    st_["h1"] = h1

                # ---- stage G(td): layer-3 (bf16) + fp8 cast ----
                if 0 <= td < T:
                    st_ = live[td]
                    m0a = m0ps_pool.tile([128, 448], F32, tag="m0a")
                    nc.tensor.matmul(m0a[:], st_["h2t"][:], w3_sb[:, 0:448],
                                     start=True, stop=True)
                    m0b = m0ps_pool.tile([128, 448], F32, tag="m0b")
                    nc.tensor.matmul(m0b[:], st_["h2t"][:], w3_sb[:, 448:896],
                                     start=True, stop=True)
                    m0_sb = m0_pool.tile([128, M0 * SPHERE], FP8)
                    nc.scalar.activation(m0_sb[:, 0:448], m0a[:],
                                         mybir.ActivationFunctionType.Copy)
                    nc.vector.tensor_copy(m0_sb[:, 448:896], m0b[:])
                    st_["m0"] = m0_sb
                    st_["out"] = out_pool.tile([128, OUTF], BF16,
                                               name="out_sb", tag="out_sb")

                # ---- silu for LN2(t2_) ----
                if 0 <= t2_ < T:
                    st_ = live[t2_]
                    h2 = h_pool.tile([128, 128], BF16, tag="h2")
                    nc.scalar.activation(h2[:], st_["ps2"][:],
                                         mybir.ActivationFunctionType.Silu,
                                         bias=nm[:, 1:2], scale=yf[:, 1:2])
                    st_["h2"] = h2

                # tr2 for tile i-5: silu2(i-5) ran last iteration
                if 0 <= i - 5 < T:
                    st_ = live[i - 5]
                    pst2 = pst_pool.tile([128, 128], BF16, tag="pst")
                    nc.tensor.transpose(pst2[:], st_["h2"][:], id_sb[:])
                    h2t = ht_pool.tile([128, 128], BF16, tag="h2t")
                    nc.vector.tensor_copy(h2t[:], pst2[:])
                    st_["h2t"] = h2t

                if 0 <= te < T:
                    rot_half(te, 1)

                # ---- output DMA for tile tf: copies ran last iteration ----
                if 0 <= tf < T:
                    nc.gpsimd.dma_start(outr[tf], live[tf]["out"][:])
                    del live[tf]

    return nc


def _envelope(d):
    e = 1.0 + (-21.0) * d ** 5 + 35.0 * d ** 6 + (-15.0) * d ** 7
    return np.where(d < 1.0, e, 0.0)


def kernel(**inputs):
    x = np.asarray(inputs["x"], np.float32)
    dist_emb = np.asarray(inputs["edge_distance_embedding"], np.float32)
    src_emb = np.asarray(inputs["source_atom_embedding"], np.float32)
    tgt_emb = np.asarray(inputs["target_atom_embedding"], np.float32)
    edge_distance = np.asarray(inputs["edge_distance"], np.float64)
    edge_index = np.asarray(inputs["edge_index"]).astype(np.int64)
    wigner = np.asarray(inputs["wigner_and_M_mapping_inv"], np.float32)
    W1 = np.asarray(inputs["W1"], np.float32)
    W2 = np.asarray(inputs["W2"], np.float32)
    W3 = np.asarray(inputs["W3"], np.float32)
    # biases/gains are zeros/ones by construction; folded out of the kernel
    for nm, triv in (("b1", 0), ("bt1", 0), ("b2", 0), ("bt2", 0), ("b3", 0),
                     ("g1", 1), ("g2", 1)):
        v = np.asarray(inputs[nm])
        assert np.all(v == triv), f"{nm} not trivial; unsupported fast path"

    srcs, tgts = edge_index[0], edge_index[1]
    scale = (_envelope(edge_distance / CUTOFF) / RESCALE * WSCALE).astype(
        np.float32)

    order = np.argsort(tgts, kind="stable")
    tsorted = tgts[order]
    starts = np.searchsorted(tsorted, np.arange(N_NODES + 1))

    # ---- pack nodes into 32-edge slots, <=3 nodes (col-groups) each ----
    core_slots = []
    max_T = 0
    for c in range(N_CORES):
        base = c * NODES_PER_CORE
        pieces = []  # (local node, start offset into its edge list, count)
        for nl in range(NODES_PER_CORE):
            d = int(starts[base + nl + 1] - starts[base + nl])
            off = 0
            while d > SLOT_E:
                pieces.append((nl, off, SLOT_E))
                off += SLOT_E
                d -= SLOT_E
            pieces.append((nl, off, d))
        pieces.sort(key=lambda p: -p[2])
        slots = []  # [rem, [(nl, off, cnt), ...]]
        for nl, off, d in pieces:
            best, bestrem = -1, SLOT_E + 1
            for i, (rem, members) in enumerate(slots):
                if len(members) < GSL and d <= rem < bestrem:
                    best, bestrem = i, rem
            if best >= 0:
                slots[best][0] -= d
                slots[best][1].append((nl, off, d))
            else:
                slots.append([SLOT_E - d, [(nl, off, d)]])
        core_slots.append(slots)
        max_T = max(max_T, -(-len(slots) // 4))

    T = max_T
    n_slots = 4 * T
    E_pad = n_slots * SLOT_E

    if T not in _CACHE:
        _CACHE[T] = _build_nc(T)
    nc = _CACHE[T]

    # ---- shared weight tensors ----
    w1_in = np.ascontiguousarray(
        (W1[:D_DIST] * W1SCALE).reshape(4, 128, 128).transpose(1, 0, 2)
        .reshape(128, 4 * 128)
    ).astype(NP_FP8)
    su_proj = (src_emb @ (W1[D_DIST:D_DIST + 128] * W1SCALE))
    tu_proj = (tgt_emb @ (W1[D_DIST + 128:] * W1SCALE))
    w2_in = W2.astype(NP_BF16)
    w3_in = W3.astype(NP_BF16)
    ident = np.eye(128, dtype=np.float32).astype(NP_BF16)

    in_maps = []
    unpack_maps = []
    for c in range(N_CORES):
        base = c * NODES_PER_CORE
        slots = core_slots[c]

        eorder = np.full(E_pad, -1, np.int64)
        grp = np.zeros(E_pad, np.int64)
        ent_t, ent_s, ent_g, ent_n = [], [], [], []
        for si, (_, members) in enumerate(slots):
            r = si * SLOT_E
            for g, (nl, off, cnt) in enumerate(members):
                s0 = starts[base + nl] + off
                eorder[r:r + cnt] = order[s0:s0 + cnt]
                grp[r:r + cnt] = g
                ent_t.append(si // 4)
                ent_s.append(si % 4)
                ent_g.append(g)
                ent_n.append(nl)
                r += cnt
        valid = eorder >= 0
        idx = eorder[valid]

        # xe gather (dist-embedding part) -> [T, 128p, 4k*128e]
        xe = np.zeros((E_pad, D_DIST), np.float32)
        xe[valid] = dist_emb[idx]
        xeT = xe.reshape(T, TILE_E, 4, 128).transpose(0, 3, 2, 1)
        # per-edge precomputed src/tgt projection, [T, 128e, 128c]
        sue = np.zeros((E_pad, 128), np.float32)
        sue[valid] = su_proj[srcs[idx]] + tu_proj[tgts[idx]]
        su_in = np.ascontiguousarray(sue.reshape(T, TILE_E, 128)).astype(
            NP_BF16)

        # group-diagonal wigner: row r of slot s, col-group g:
        # xw[t, 32s+r, 768 + m*147 + g*49 + f] = wig[e,f,m]*scale
        wrows = np.zeros((E_pad, M0, LFULL), np.float32)
        wrows[valid] = (
            wigner[idx, :, :M0] * scale[idx][:, None, None]
        ).transpose(0, 2, 1)
        spos = (np.arange(E_pad) // SLOT_E) % 2
        wsec = np.zeros((E_pad, M0, 2, GSL, LFULL), np.float32)
        wsec[np.arange(E_pad), :, spos, grp, :] = wrows
        wsec = wsec.reshape(T, 128, WCOLS)

        xw_in = np.ascontiguousarray(np.concatenate(
            (xeT.reshape(T, 128, 512), wsec), axis=2,
        )).astype(NP_FP8)

        in_maps.append({
            "xw": xw_in, "su": su_in,
            "w1": w1_in, "w2": w2_in, "w3": w3_in, "ident": ident,
        })
        unpack_maps.append((np.asarray(ent_t), np.asarray(ent_s),
                            np.asarray(ent_g), np.asarray(ent_n)))

    global LAST
    res = run_bass_kernel_spmd(
        nc, in_maps, core_ids=list(range(N_CORES)), trace=TRACE, **TRACE_KW
    )
    LAST = res

    out = np.empty((N_NODES, LFULL, SPHERE), np.float32)
    inv_ws = np.float32(1.0 / WSCALE)
    for c in range(N_CORES):
        r = res.results[c]
        # [T, 128c, 4s, 3g, 49f] -> [T, 4, 3, 49, 128]
        o = (np.asarray(r["outr"], np.float32) * inv_ws).reshape(
            T, 128, 4, GSL, LFULL).transpose(0, 2, 3, 4, 1)
        tt, ss, gg, nn = unpack_maps[c]
        acc = np.zeros((NODES_PER_CORE, LFULL, SPHERE), np.float32)
        np.add.at(acc, nn, o[tt, ss, gg])
        out[c * NODES_PER_CORE:(c + 1) * NODES_PER_CORE] = (
            x[c * NODES_PER_CORE:(c + 1) * NODES_PER_CORE] + acc
        )
    return out
